# revision 27
# baseline (speedup 1.0000x reference)
"""Trainium2 Bass kernel for nn_Affinity: M = relu(Xh ⊕ Yh + b1) @ W2 + b2.

Math (reference):
    Xh = X @ (W1[:, :C] @ Wsr).T          # [N1, H]
    Yh = Y @ (W1[:, C:] @ Wtg).T          # [N2, H]
    M[a, b] = sum_h W2[h] * relu(Xh[a, h] + Yh[b, h] + b1[h]) + b2

Sharding: rows of X (N1) split across 8 cores; each core computes a
[128, 1024] tile of M. No cross-core communication.

Per-core kernel design:
  - Host pre-folds weights: AxT = (W1[:,:C] @ Wsr).T, AyT = (W1[:,C:] @ Wtg).T.
  - PE computes XhT [h, a] and YhT [h, b] (h on partitions).
  - Main loop over a: V = relu(YhT + XhT[:, a]) via DVE tensor_scalar
    (bf16, 4x mode) and ACT activation (bias trick), split ~3:1.
  - Contraction over h via PE: lhsT is a sliding one-hot window holding
    W2 in the column matching row a, so each matmul accumulates row a of
    the output into PSUM at partition a. MODE:
      "full": M=128 windows, standard 128x128 array mode (serial PE).
      "col4": M=32 windows, 128x32 column-tiled mode (4 concurrent
              streams); every matmul in the program is col-tiled so the
              array mode never switches.
  - PSUM [128, 512] x2 evacuated once at the end (+b2) and DMA'd out.
"""

import sys

if "/opt/trn_rl_repo" not in sys.path:
    sys.path.insert(0, "/opt/trn_rl_repo")

import numpy as np
import ml_dtypes

import concourse.bass as bass
import concourse.bacc as bacc
import concourse.tile as tile
from concourse import mybir
from concourse.bass_utils import run_bass_kernel_spmd

N1, N2, C, H = 1024, 1024, 128, 256
NCORES = 8
P = N1 // NCORES  # 128 rows of X per core

F32 = mybir.dt.float32
BF16 = mybir.dt.bfloat16
BF16_NP = ml_dtypes.bfloat16

MODE = "col4"  # "full" | "col4"
# Measured per-V-tile cost (ns) on HW; used for greedy load balancing of
# the 256 V-tiles across the three elementwise-capable engines.
V_COST = {"D": 432, "A": 1250, "G": 1100}
USE_GPSIMD = False

_CACHE = {}

# One-hot window width: window buffer has W2 at column WMAX-1, zeros
# elsewhere; slice [WMAX-1-m : WMAX-1-m+M] puts W2 at local column m.
def _wmax():
    return 255 if MODE == "full" else 63


# Packed-input layout (single DMA): one [128, PACK_W] f32 tensor.
# Columns: xt[0:128] | yt[128:1152] | axt[1152:1408] | ayt[1408:1664] |
# b1t0[1664] | b1t1[1665] | b2[1666] | zw0 bf16x64 [1667:1699] |
# zw1 bf16x64 [1699:1731]   (zw slices bitcast to bf16 on device)
PACK_W = 1731


def _build_program():
    nc = bacc.Bacc("TRN2", debug=False)

    pack = nc.dram_tensor("pack", [C, PACK_W], F32, kind="ExternalInput")
    m_out = nc.dram_tensor("m_out", [P, N2], F32, kind="ExternalOutput")

    AL = mybir.AluOpType

    with tile.TileContext(nc) as tc:
        with (
            tc.tile_pool(name="const", bufs=1) as const,
            tc.tile_pool(name="v", bufs=16) as vpool,
            tc.tile_pool(name="outp", bufs=2) as outp,
        ):
            pk = const.tile([C, PACK_W], F32)
            nc.sync.dma_start(pk[:], pack[:])
            xt_sb = pk[:, 0:128]
            yt_sb = pk[:, 128:1152]
            axt_sb = pk[:, 1152:1408]
            ayt_sb = pk[:, 1408:1664]
            b1_sb = [pk[:, 1664:1665], pk[:, 1665:1666]]
            b2_sb = pk[:, 1666:1667]
            zw_sb = [
                pk[:, 1667:1699].bitcast(BF16),
                pk[:, 1699:1731].bitcast(BF16),
            ]

            def prep_matmul(ps_ap, lhsT_ap, rhs_ap):
                # In col4 mode every matmul must be 128x32 col-tiled so
                # the PE array mode never switches mid-kernel.
                if MODE == "col4":
                    mtot = lhsT_ap.shape[1]
                    for mo in range(0, mtot, 32):
                        jj = (ps_ap.base_partition() + mo) // 32
                        nc.tensor.matmul(
                            ps_ap[mo : mo + 32, :],
                            lhsT_ap[:, mo : mo + 32],
                            rhs_ap,
                            start=True, stop=True,
                            tile_position=(0, 32 * (jj % 4)),
                        )
                else:
                    nc.tensor.matmul(
                        ps_ap, lhsT_ap, rhs_ap, start=True, stop=True
                    )

            # Prep phase uses its own PSUM pool, released before the main
            # loop (which needs all 8 banks in col4 mode).
            with tc.tile_pool(name="pst", bufs=2, space="PSUM") as pst:
                # XhT [h, a] per h-tile, with b1 folded in (f32: ACT bias
                # and DVE tensor_scalar per-partition operand must be f32).
                xhb_f32 = []
                for t in range(2):
                    ps = pst.tile([C, P], F32, tag="prep", name=f"ps_xh{t}")
                    prep_matmul(ps[:], axt_sb[:, t * 128 : (t + 1) * 128], xt_sb[:])
                    xf = const.tile([C, P], F32, tag=f"xhb_f32_{t}", name=f"xhb{t}")
                    nc.vector.tensor_scalar_add(xf[:], ps[:], b1_sb[t][:, 0:1])
                    xhb_f32.append(xf)

                # YhT [h, b] per h-tile, bf16 (b1 folded into Xh side).
                # PSUM evacuation on ACT (ScalarE is closest to PSUM).
                yh = []
                for t in range(2):
                    ysb = const.tile([C, N2], BF16, tag=f"yh_{t}", name=f"yh{t}")
                    for half in range(2):
                        ps = pst.tile(
                            [C, 512], F32, tag="prep", name=f"ps_yh{t}{half}"
                        )
                        prep_matmul(
                            ps[:],
                            ayt_sb[:, t * 128 : (t + 1) * 128],
                            yt_sb[:, half * 512 : (half + 1) * 512],
                        )
                        if half == 0:
                            nc.vector.tensor_copy(
                                ysb[:, half * 512 : (half + 1) * 512], ps[:]
                            )
                        else:
                            nc.scalar.copy(
                                ysb[:, half * 512 : (half + 1) * 512], ps[:]
                            )
                    yh.append(ysb)

            with tc.tile_pool(name="pso", bufs=1, space="PSUM") as pso:
                if MODE == "col4":
                    # One PSUM bank per (col-group, half): each accumulation
                    # region exclusively owns a bank, so per-region
                    # start=True bank-clears are safe.
                    ps_out = [
                        [
                            pso.tile(
                                [128, 512], F32,
                                tag=f"pso_{j}_{h}", name=f"ps_out_{j}_{h}",
                            )
                            for h in range(2)
                        ]
                        for j in range(4)
                    ]
                else:
                    ps_out = [
                        pso.tile([128, 512], F32, tag=f"pso_{h}", name=f"ps_out_{h}")
                        for h in range(2)
                    ]

                # a-iteration order: in col4 mode group a's so consecutive
                # matmuls rotate through the 4 column groups.
                if MODE == "col4":
                    a_order = [32 * j + g for g in range(32) for j in range(4)]
                else:
                    a_order = list(range(128))
                a_chunk = 4

                # Greedy least-loaded assignment of V-tiles to engines.
                load = {"D": 0.0, "A": 0.0, "G": 0.0 if USE_GPSIMD else 1e18}

                def v_engine():
                    e = min(load, key=lambda k: load[k] + V_COST[k])
                    load[e] += V_COST[e]
                    return e

                first_a, last_a = a_order[0], a_order[-1]
                for ci in range(0, 128, a_chunk):
                    chunk = a_order[ci : ci + a_chunk]
                    vs = {}
                    for t in range(2):
                        for a in chunk:
                            v = vpool.tile([C, N2], BF16, tag="v", name=f"v_{t}_{a}")
                            eng = v_engine()
                            if eng == "A":
                                nc.scalar.activation(
                                    v[:], yh[t][:],
                                    mybir.ActivationFunctionType.Relu,
                                    bias=xhb_f32[t][:, a : a + 1],
                                )
                            else:
                                veng = nc.vector if eng == "D" else nc.gpsimd
                                veng.tensor_scalar(
                                    v[:], yh[t][:],
                                    xhb_f32[t][:, a : a + 1], 0.0,
                                    AL.add, AL.max,
                                )
                            vs[(t, a)] = v
                    for t in range(2):
                        for half in range(2):
                            for a in chunk:
                                if MODE == "col4":
                                    j, m = a // 32, a % 32
                                    nc.tensor.matmul(
                                        ps_out[j][half][32 * j : 32 * j + 32, :],
                                        zw_sb[t][:, 31 - m : 63 - m],
                                        vs[(t, a)][:, half * 512 : (half + 1) * 512],
                                        start=(m == 0 and t == 0),
                                        stop=(m == 31 and t == 1),
                                        skip_group_check=True,
                                        tile_position=(0, 32 * j),
                                    )
                                else:
                                    nc.tensor.matmul(
                                        ps_out[half][:, :],
                                        zw_sb[t][:, 127 - a : 255 - a],
                                        vs[(t, a)][:, half * 512 : (half + 1) * 512],
                                        start=(a == first_a and t == 0),
                                        stop=(a == last_a and t == 1),
                                        skip_group_check=True,
                                    )

                for half in range(2):
                    o = outp.tile([128, 512], F32, tag="o", name=f"o_{half}")
                    if MODE == "col4":
                        for j in range(4):
                            sl = slice(32 * j, 32 * j + 32)
                            if j % 2 == 0:
                                nc.vector.tensor_scalar_add(
                                    o[sl, :], ps_out[j][half][sl, :], b2_sb[sl, 0:1]
                                )
                            else:
                                nc.scalar.activation(
                                    o[sl, :], ps_out[j][half][sl, :],
                                    mybir.ActivationFunctionType.Identity,
                                    bias=b2_sb[sl, 0:1],
                                )
                    else:
                        nc.vector.tensor_scalar_add(
                            o[:], ps_out[half][:], b2_sb[:, 0:1]
                        )
                    nc.sync.dma_start(m_out[:, half * 512 : (half + 1) * 512], o[:])

    nc.compile()
    return nc


def _get_program():
    if "nc" not in _CACHE:
        _CACHE["nc"] = _build_program()
    return _CACHE["nc"]


def kernel(X, Y, Wsr, Wtg, W1, b1, W2, b2, _trace=False, _trace_kwargs=None):
    X = np.asarray(X, np.float32)
    Y = np.asarray(Y, np.float32)
    Wsr = np.asarray(Wsr, np.float32)
    Wtg = np.asarray(Wtg, np.float32)
    W1 = np.asarray(W1, np.float32)
    b1 = np.asarray(b1, np.float32)
    W2 = np.asarray(W2, np.float32)
    b2 = np.asarray(b2, np.float32)

    # Host-side weight folding (tiny: O(C^2 H)).
    AxT = np.ascontiguousarray((W1[:, :C] @ Wsr).T)  # [C, H]
    AyT = np.ascontiguousarray((W1[:, C:] @ Wtg).T)  # [C, H]
    Zw = np.zeros((2, C, 64), BF16_NP)
    Zw[0, :, 31] = W2[0, :C].astype(BF16_NP)
    Zw[1, :, 31] = W2[0, C:].astype(BF16_NP)
    b2v = np.full((P, 1), b2[0], np.float32)
    XT = np.ascontiguousarray(X.T)  # [C, N1]
    YT = np.ascontiguousarray(Y.T)  # [C, N2]

    common = np.concatenate(
        [
            YT, AxT, AyT,
            b1[:C, None], b1[C:, None], b2v,
            Zw[0].view(np.float32), Zw[1].view(np.float32),
        ],
        axis=1,
    ).astype(np.float32)

    in_maps = [
        {
            "pack": np.ascontiguousarray(
                np.concatenate([XT[:, c * P : (c + 1) * P], common], axis=1)
            ),
        }
        for c in range(NCORES)
    ]

    nc = _get_program()
    res = run_bass_kernel_spmd(
        nc, in_maps, list(range(NCORES)), trace=_trace,
        **(_trace_kwargs or {}),
    )
    _CACHE["last_results"] = res
    M = np.concatenate([res.results[c]["m_out"] for c in range(NCORES)], axis=0)
    return M.astype(np.float32)


# revision 29
# speedup vs baseline: 1.0406x; 1.0406x over previous
"""Trainium2 Bass kernel for nn_Affinity: M = relu(Xh (+) Yh + b1) @ W2 + b2.

Math (reference):
    Xh = X @ (W1[:, :C] @ Wsr).T          # [N1, H]
    Yh = Y @ (W1[:, C:] @ Wtg).T          # [N2, H]
    M[a, b] = sum_h W2[h] * relu(Xh[a, h] + Yh[b, h] + b1[h]) + b2

Sharding: rows of X (N1=1024) split across 8 cores; each core computes a
[128, 1024] tile of M; no cross-core communication.

Per-core design (raw bacc, hand-placed semaphores):
  - Host pre-folds weights (AxT, AyT) and packs all inputs into one
    [128, 1731] f32 tensor (2 DMAs; bf16 one-hot W2 windows bitcast).
  - PE warm-up matmuls on a zero scratch during the input DMA window so
    the HAM clock-gate reaches 2.4 GHz before prep.
  - PE computes XhT [h, a] and YhT [h, b] (h on partitions); DVE/ACT
    evacuate PSUM (xhb gets b1 folded in; yh stored bf16).
  - Main loop over 256 V-tiles (a, h-tile): V = relu(YhT + XhT[:, a])
    via DVE tensor_scalar (bf16 SBUF 4x mode, ~396 ns/tile, 186 tiles)
    and ACT activation Relu-with-bias (~1040 ns/tile, 70 tiles),
    greedy-balanced.
  - Contraction over h on PE: lhsT is a sliding one-hot window holding
    W2 in the column matching row a, so each matmul accumulates output
    row a into PSUM partition a; 128x32 column-tiled (4 concurrent
    streams), one PSUM bank per (col-group, b-half) region.
  - Early per-region evacuation overlapped with the last matmuls, then
    2 output DMAs. All sync is fused sem waits + then_inc piggybacks;
    no Tile framework, no block-exit barrier.
"""

import sys

if "/opt/trn_rl_repo" not in sys.path:
    sys.path.insert(0, "/opt/trn_rl_repo")

import numpy as np
import ml_dtypes

import concourse.bacc as bacc
from concourse import mybir

N1, N2, C, H = 1024, 1024, 128, 256
NCORES = 8
P = N1 // NCORES

F32 = mybir.dt.float32
BF16 = mybir.dt.bfloat16
BF16_NP = ml_dtypes.bfloat16

NBUF = 24  # V-tile ring slots
V_COST = {"D": 396, "A": 1040}

# Packed-input layout (two DMAs): [128, PACK_W] f32.
# cols: xt[0:128] | axt[128:384] | ayt[384:640] | b1t0[640] | b1t1[641] |
#       b2[642] | zw0 (64 bf16 = 32 f32) [643:675] | zw1 [675:707] |
#       yt [707:1731]
PACK_W = 1731

_CACHE = {}


def _schedule():
    """Global V-tile order + greedy engine assignment.

    Returns (tiles, eng) where tiles[i] = (t, a) in production order and
    eng[i] in {"D", "A"}."""
    a_order = [32 * j + g for g in range(32) for j in range(4)]
    tiles = []
    for ci in range(0, 128, 4):
        chunk = a_order[ci : ci + 4]
        for t in range(2):
            for a in chunk:
                tiles.append((t, a))
    load = {"D": 0.0, "A": 0.0}
    eng = []
    for _ in tiles:
        e = min(load, key=lambda k: load[k] + V_COST[k])
        load[e] += V_COST[e]
        eng.append(e)
    return tiles, eng


def _build_program():
    nc = bacc.Bacc("TRN2", debug=False)
    AL = mybir.AluOpType
    AF = mybir.ActivationFunctionType

    pack = nc.dram_tensor("pack", [C, PACK_W], F32, kind="ExternalInput")
    m_out = nc.dram_tensor("m_out", [P, N2], F32, kind="ExternalOutput")

    pk = nc.alloc_sbuf_tensor("pk", [C, PACK_W], F32).ap()
    xt_sb = pk[:, 0:128]
    axt_sb = pk[:, 128:384]
    ayt_sb = pk[:, 384:640]
    b1_sb = [pk[:, 640:641], pk[:, 641:642]]
    b2_sb = pk[:, 642:643]
    zw_sb = [
        pk[:, 643:675].bitcast(BF16),
        pk[:, 675:707].bitcast(BF16),
    ]
    yt_sb = pk[:, 707:1731]

    yh = [nc.alloc_sbuf_tensor(f"yh{t}", [C, N2], BF16).ap() for t in range(2)]
    xhb = [nc.alloc_sbuf_tensor(f"xhb{t}", [C, P], F32).ap() for t in range(2)]
    vsl = [
        nc.alloc_sbuf_tensor(f"v{s}", [C, N2], BF16).ap() for s in range(NBUF)
    ]
    osb = [nc.alloc_sbuf_tensor(f"o{h}", [128, 512], F32).ap() for h in range(2)]
    warm = nc.alloc_sbuf_tensor("warm", [128, 512], BF16).ap()

    # 8 PSUM banks. Prep reuses banks 0-5 (xh in 0-1, yh in 2-5); main
    # regions (j, half) own bank 2j+half. Reuse guarded by act_prep wait.
    pso = [nc.alloc_psum_tensor(f"pso{b}", [128, 512], F32).ap() for b in range(8)]

    sem = {
        name: nc.alloc_semaphore(name)
        for name in (
            "dma_in", "dma_in2", "pe_prep", "prep_d", "prep_a", "v_d", "v_a", "v_free",
            "fin", "evac_d", "evac_a", "dma_out", "warm",
        )
    }

    tiles, eng = _schedule()
    # For tile i: its producer-engine count up to and including i.
    nd = na = 0
    prod_count = []
    for e in eng:
        if e == "D":
            nd += 1
            prod_count.append(nd)
        else:
            na += 1
            prod_count.append(na)
    tile_index = {tv: i for i, tv in enumerate(tiles)}

    if True:  # direct emission, no Block exit barrier

        def _body_gp(gp):
            gp.memset(warm, 0.0).then_inc(sem["warm"], 1)

        def _body_sync(sync):
            sync.dma_start(pk[:, 0:707], pack[:, 0:707]).then_inc(sem["dma_in"], 16)
            sync.dma_start(pk[:, 707:1731], pack[:, 707:1731]).then_inc(
                sem["dma_in2"], 16
            )
            for half in range(2):
                sync.wait_ge(sem["evac_d"], 2 * (half + 1))
                sync.wait_ge(sem["evac_a"], 2 * (half + 1))
                sync.dma_start(
                    m_out[:, half * 512 : (half + 1) * 512], osb[half][:, :]
                ).then_inc(sem["dma_out"], 16)
            sync.wait_ge(sem["dma_out"], 32)

        def _body_pe(pe):
            pe.wait_ge(sem["warm"], 1)
            for w in range(8):
                pe.matmul(
                    pso[7][96:128, :],
                    warm[:, 0:32],
                    warm[:, :],
                    start=True, stop=True,
                    skip_group_check=True,
                    tile_position=(0, 96),
                )
            pe.wait_ge(sem["dma_in"], 16)
            # prep XhT: 2 t-tiles x 4 col-chunks into banks 0/1
            for t in range(2):
                for mo in range(0, 128, 32):
                    ins = pe.matmul(
                        pso[t][mo : mo + 32, 0:128],
                        axt_sb[:, t * 128 + mo : t * 128 + mo + 32],
                        xt_sb,
                        start=True, stop=True,
                        tile_position=(0, mo),
                    )
                    if mo == 96:
                        ins.then_inc(sem["pe_prep"], 1)
            pe.wait_ge(sem["dma_in2"], 16)
            # prep YhT: (t, half) -> bank 2+2t+half
            for t in range(2):
                for half in range(2):
                    for mo in range(0, 128, 32):
                        ins = pe.matmul(
                            pso[2 + 2 * t + half][mo : mo + 32, :],
                            ayt_sb[:, t * 128 + mo : t * 128 + mo + 32],
                            yt_sb[:, half * 512 : (half + 1) * 512],
                            start=True, stop=True,
                            tile_position=(0, mo),
                        )
                        if mo == 96:
                            ins.then_inc(sem["pe_prep"], 1)
            # wait until DVE/ACT consumed all prep psum (banks reused below)
            pe.wait_ge(sem["prep_d"], 3)
            pe.wait_ge(sem["prep_a"], 3)
            n_tiles = len(tiles)
            for ci in range(0, n_tiles, 8):
                for t in range(2):
                    for half in range(2):
                        for k in range(4):
                            i = ci + 4 * t + k
                            tt, a = tiles[i]
                            assert tt == t
                            j, m = a // 32, a % 32
                            if half == 0:
                                vs = sem["v_d"] if eng[i] == "D" else sem["v_a"]
                                pe.wait_ge(vs, prod_count[i])
                            ins = pe.matmul(
                                pso[2 * j + half][32 * j : 32 * j + 32, :],
                                zw_sb[t][:, 31 - m : 63 - m],
                                vsl[i % NBUF][:, half * 512 : (half + 1) * 512],
                                start=(m == 0 and t == 0),
                                stop=(m == 31 and t == 1),
                                skip_group_check=True,
                                tile_position=(0, 32 * j),
                            )
                            last_chunk = ci == n_tiles - 8
                            if last_chunk and t == 1:
                                # fin counts the final 8 region-completing
                                # MMs (t1h0 j0-3 then t1h1 j0-3): region
                                # (j, h) is final after fin >= 4*h + j + 1
                                ins.then_inc(sem["fin"], 1)
                            elif half == 1 and i < n_tiles - 8:
                                ins.then_inc(sem["v_free"], 1)

        def _body_act(act):
            act.wait_ge(sem["pe_prep"], 2)
            act.activation(
                xhb[1], pso[1][:, 0:128], AF.Identity, bias=b1_sb[1][:, 0:1]
            ).then_inc(sem["prep_a"], 1)
            for t in range(2):
                half = 1
                act.wait_ge(sem["pe_prep"], 3 + 2 * t + half)
                act.activation(
                    yh[t][:, half * 512 : (half + 1) * 512],
                    pso[2 + 2 * t + half][:, :],
                    AF.Copy,
                ).then_inc(sem["prep_a"], 1)
            # own writes visible + cross-engine prep from DVE
            act.wait_ge(sem["prep_a"], 3)
            act.wait_ge(sem["prep_d"], 3)
            for i, (t, a) in enumerate(tiles):
                if eng[i] != "A":
                    continue
                if i >= NBUF:
                    act.wait_ge(sem["v_free"], i - NBUF + 1)
                act.activation(
                    vsl[i % NBUF], yh[t], AF.Relu, bias=xhb[t][:, a : a + 1]
                ).then_inc(sem["v_a"], 1)
            for half in range(2):
                for j in (1, 3):
                    sl = slice(32 * j, 32 * j + 32)
                    act.wait_ge(sem["fin"], 4 * half + j + 1)
                    act.activation(
                        osb[half][sl, :], pso[2 * j + half][sl, :],
                        AF.Identity, bias=b2_sb[sl, 0:1],
                    ).then_inc(sem["evac_a"], 1)

        def _body_dve(dve):
            dve.wait_ge(sem["pe_prep"], 1)
            dve.tensor_scalar_add(
                xhb[0], pso[0][:, 0:128], b1_sb[0][:, 0:1]
            ).then_inc(sem["prep_d"], 1)
            for t in range(2):
                half = 0
                dve.wait_ge(sem["pe_prep"], 3 + 2 * t + half)
                dve.tensor_copy(
                    yh[t][:, half * 512 : (half + 1) * 512],
                    pso[2 + 2 * t + half][:, :],
                ).then_inc(sem["prep_d"], 1)
            dve.wait_ge(sem["prep_d"], 3)
            dve.wait_ge(sem["prep_a"], 3)
            for i, (t, a) in enumerate(tiles):
                if eng[i] != "D":
                    continue
                if i >= NBUF:
                    dve.wait_ge(sem["v_free"], i - NBUF + 1)
                dve.tensor_scalar(
                    vsl[i % NBUF], yh[t], xhb[t][:, a : a + 1], 0.0,
                    AL.add, AL.max,
                ).then_inc(sem["v_d"], 1)
            for half in range(2):
                for j in (0, 2):
                    sl = slice(32 * j, 32 * j + 32)
                    dve.wait_ge(sem["fin"], 4 * half + j + 1)
                    dve.tensor_scalar_add(
                        osb[half][sl, :], pso[2 * j + half][sl, :], b2_sb[sl, 0:1]
                    ).then_inc(sem["evac_d"], 1)

        _body_gp(nc.gpsimd)
        _body_sync(nc.sync)
        _body_pe(nc.tensor)
        _body_act(nc.scalar)
        _body_dve(nc.vector)

    nc.compile()
    return nc


def _get_program():
    if "nc" not in _CACHE:
        _CACHE["nc"] = _build_program()
    return _CACHE["nc"]


def make_in_maps(X, Y, Wsr, Wtg, W1, b1, W2, b2):
    AxT = np.ascontiguousarray((W1[:, :C] @ Wsr).T)
    AyT = np.ascontiguousarray((W1[:, C:] @ Wtg).T)
    Zw = np.zeros((2, C, 64), BF16_NP)
    Zw[0, :, 31] = W2[0, :C].astype(BF16_NP)
    Zw[1, :, 31] = W2[0, C:].astype(BF16_NP)
    b2v = np.full((P, 1), b2[0], np.float32)
    XT = np.ascontiguousarray(X.T)
    YT = np.ascontiguousarray(Y.T)

    common = np.concatenate(
        [
            AxT, AyT, b1[:C, None], b1[C:, None], b2v,
            Zw[0].view(np.float32), Zw[1].view(np.float32), YT,
        ],
        axis=1,
    ).astype(np.float32)
    return [
        {
            "pack": np.ascontiguousarray(
                np.concatenate([XT[:, c * P : (c + 1) * P], common], axis=1)
            )
        }
        for c in range(NCORES)
    ]


def kernel(X, Y, Wsr, Wtg, W1, b1, W2, b2, _trace=False, _trace_kwargs=None):
    from concourse.bass_utils import run_bass_kernel_spmd

    args = [np.asarray(v, np.float32) for v in (X, Y, Wsr, Wtg, W1, b1, W2, b2)]
    in_maps = make_in_maps(*args)
    nc = _get_program()
    res = run_bass_kernel_spmd(
        nc, in_maps, list(range(NCORES)), trace=_trace, **(_trace_kwargs or {})
    )
    _CACHE["last_results"] = res
    M = np.concatenate([res.results[c]["m_out"] for c in range(NCORES)], axis=0)
    return M.astype(np.float32)
